# revision 42
# baseline (speedup 1.0000x reference)
# MoE layer (top-2 routing, degenerate capacity C=2) on 8 TRN2 NeuronCores.
#
# Math (reference collapses over the capacity axis since the dispatch mask is
# broadcast identically into both capacity slots):
#   scores = softmax(x @ Wg + bg)                      [G,S,E]
#   top-2 per token -> dm (0/1 mask), cw = 2 * softmax(top2 scores) scattered
#   D[e,g,:]  = sum_s dm[g,s,e] * x[g,s,:]             (dispatch, per group)
#   h[e,g,:]  = silu(D[e,g,:] @ wi[e].T)
#   eo[e,g,:] = h[e,g,:] @ wo[e].T
#   out[g,s,:] = sum_e cw[g,s,e] * eo[e,g,:]
#
# Sharding: core c owns group g=c for gating/dispatch/combine and expert e=c
# for the FFN. One AllToAll redistributes the dispatched [8,M] rows into
# expert-parallel layout; four small AllToAlls (one per 512-column FFN2
# chunk) bring expert outputs back so each chunk's combine hides under the
# still-running weight stream.
#
# The kernel is HBM-bandwidth bound (the FFN streams 67 MB of bf16 weights per
# core to multiply a [8,M] activation), so everything is organized around
# keeping the DMA engines saturated end-to-end:
#   - x is loaded once in fp32 (gating needs fp32 top-2 selection); the bf16
#     copy for the dispatch matmul is cast on-chip by the otherwise-idle
#     GpSimd engine instead of a second HBM load.
#   - wi/wo are pre-packed on the host so each FFN chunk is one large
#     contiguous-per-partition DMA (2 MB per dma_start).
#   - x and wi loads are issued on nc.sync in that order (HWDGE executes
#     FIFO per issuing engine, so x has strict priority during gating and wi
#     streams the moment x is done). The first FFN2 chunk's wo loads are
#     issued from nc.scalar so they fill the DMA gap while the dispatch
#     AllToAll blocks FFN1.
#   - per-engine instruction streams are software-pipelined: work that
#     depends on a tile's late vector-chain is emitted after the next tile's
#     matmuls so the PE never stalls mid-phase.
#   - the output is stored in bf16 (cast back to fp32 on the host), halving
#     the store traffic.

import os
from contextlib import ExitStack

import numpy as np
import ml_dtypes

import concourse.bass as bass
from concourse import bacc
import concourse.mybir as mybir
import concourse.tile as tile
from concourse.bass import ts
from concourse.masks import make_identity

F32 = mybir.dt.float32
BF16 = mybir.dt.bfloat16
AF = mybir.ActivationFunctionType
ALU = mybir.AluOpType
AX = mybir.AxisListType

P = 128

# Full problem dims (hardcoded per harness contract)
G_FULL, S_FULL, M_FULL, H_FULL, E_FULL = 8, 2048, 2048, 8192, 8
N_CORES = 8

LAST_RESULT = None  # BassKernelResults of the most recent device run (for test.py)


def build_bass(S=S_FULL, M=M_FULL, H=H_FULL, E=E_FULL, n_cores=N_CORES):
    assert E == n_cores, "AllToAll layout assumes E == n_cores"
    SB, MO, HB = S // P, M // P, H // P
    HCH = 512                  # FFN1 matmul free-dim chunk along H
    HC = H // HCH              # 16 FFN1 chunks, one 2 MB DMA each
    MCH = 512                  # FFN2 / combine / output m-chunk
    MC = M // MCH              # 4 FFN2 chunks
    WOQ = 16                   # ho-blocks per wo quarter-DMA (2 MB each)
    NQ = HB // WOQ             # 4 wo DMAs per FFN2 chunk
    DCH = 512                  # dispatch matmul free-dim chunk
    DC = M // DCH

    nc = bacc.Bacc(num_devices=n_cores)
    rg = [list(range(n_cores))]

    xg = nc.declare_dram_parameter("xg", [S, M], F32, False)
    wg = nc.declare_dram_parameter("wg", [P, MO, E], F32, False)
    bgp = nc.declare_dram_parameter("bgp", [1, E], F32, False)
    # wiP[p, hc, mo, c] = wi[e][hc*HCH + c, mo*P + p]  (flattened to [P, ...])
    wiP = nc.declare_dram_parameter("wiP", [P, HC * MO * HCH], BF16, False)
    # woP[p, mc, ho, c] = wo[e][mc*MCH + c, ho*P + p]
    woP = nc.declare_dram_parameter("woP", [P, MC * HB * MCH], BF16, False)
    out = nc.declare_dram_parameter("out", [S, M], BF16, True)

    with tile.TileContext(nc) as tc, ExitStack() as stack:
        # ---------- persistent pools ----------
        const_pool = stack.enter_context(tc.tile_pool(name="const", bufs=1))
        ident_f = const_pool.tile([P, P], F32, name="ident_f")
        make_identity(nc, ident_f)
        ident_b = const_pool.tile([P, P], BF16, name="ident_b")
        nc.vector.tensor_copy(ident_b[:], ident_f[:])
        ones1 = const_pool.tile([1, P], F32, name="ones1")
        nc.vector.memset(ones1[:], 1.0)
        wg_sb = const_pool.tile([P, MO, E], F32, name="wg_sb")
        nc.scalar.dma_start(wg_sb[:], wg[:])
        bg_sb = const_pool.tile([1, E], F32, name="bg_sb")
        nc.scalar.dma_start(bg_sb[:], bgp[:])

        keep_pool = stack.enter_context(tc.tile_pool(name="keep", bufs=1))
        cwT_sb = keep_pool.tile([E, SB, P], BF16, name="cwT_sb")
        dt_sb = keep_pool.tile([P, MO, E], BF16, name="dt_sb")
        ht_sb = keep_pool.tile([P, HB, E], BF16, name="ht_sb")
        eo_sb = keep_pool.tile([E, M], BF16, name="eo_sb")

        # wi streaming pool: 6 x 2 MB chunks in flight (the DMAs queue on
        # nc.sync behind x, so x keeps strict priority during gating).
        wi_pool = stack.enter_context(tc.tile_pool(name="wi", bufs=5))

        dram = stack.enter_context(tc.tile_pool(name="dram", bufs=1, space="DRAM"))
        d_in = dram.tile([E, M], BF16, name="d_in")
        d_out = dram.tile([E, M], BF16, name="d_out")
        eo_in = [dram.tile([E, MCH], BF16, name=f"eo_in{i}") for i in range(MC)]
        eo_out = [dram.tile([E, MCH], BF16, name=f"eo_out{i}") for i in range(MC)]

        # ---------- phase A: gating + dispatch (group-parallel) ----------
        with (
            tc.tile_pool(name="psA", bufs=2, space="PSUM") as psA,
            tc.tile_pool(name="psD", bufs=1, space="PSUM") as psD,
        ):
            d_ps = psD.tile([E, M], F32, name="d_ps")
            with (
                tc.tile_pool(name="xa", bufs=2) as xa,
                tc.tile_pool(name="xt", bufs=2) as xt,
                tc.tile_pool(name="sp", bufs=2) as sp,
            ):
                # Absorb identity (gpsimd) and wg (DMA lane) ticks into PE's
                # vector clock so later matmuls carry at most one sem wait
                # each (walrus limits sync waits per PE instruction).
                ptd = psA.tile([P, P], F32, tag="pst", bufs=3, name="ptd")
                nc.tensor.transpose(ptd[:], ident_f[:], ident_f[:])
                dmy0 = psA.tile([E, E], F32, tag="score", bufs=1, name="dmy0")
                nc.tensor.matmul(dmy0[:], lhsT=wg_sb[:, 0, :], rhs=wg_sb[:, 0, :], start=True, stop=True)

                # Deferred-emission slot: tile sb's cw-transpose + dispatch
                # matmuls are emitted after tile sb+1's transposes/scores so
                # the PE stream never waits on the gating vector chain.
                pend = None
                for sb in range(SB):
                    x_t = xa.tile([P, M], F32, tag="x", bufs=6, name=f"x{sb}")
                    nc.sync.dma_start(x_t[:], xg[ts(sb, P), :])
                    # bf16 copy for the dispatch matmul — cast on-chip on the
                    # otherwise-idle GpSimd engine.
                    x_bf = xa.tile([P, M], BF16, tag="xbf", bufs=3, name=f"xbf{sb}")
                    nc.gpsimd.tensor_copy(x_bf[:], x_t[:])
                    xT_t = xt.tile([P, M], F32, tag="xT", name=f"xT{sb}")
                    score_ps = psA.tile([P, E], F32, tag="score", bufs=1, name=f"score{sb}")
                    # all transposes first, score matmuls after: the PE rides
                    # ahead of the PSUM->SBUF copies instead of waiting for
                    # each copy before the next tiny score matmul
                    for mo in range(MO):
                        pt = psA.tile([P, P], F32, tag="pst", bufs=3, name=f"pt{sb}_{mo}")
                        nc.tensor.transpose(pt[:], x_t[:, ts(mo, P)], ident_f[:])
                        if mo % 2 == 0:
                            nc.vector.tensor_copy(xT_t[:, ts(mo, P)], pt[:])
                        else:
                            nc.scalar.copy(xT_t[:, ts(mo, P)], pt[:])
                    for mo in range(MO):
                        nc.tensor.matmul(
                            score_ps[:], lhsT=xT_t[:, ts(mo, P)], rhs=wg_sb[:, mo, :],
                            start=(mo == 0), stop=False,
                        )
                    nc.tensor.matmul(
                        score_ps[:], lhsT=ones1[:], rhs=bg_sb[:], start=False, stop=True,
                    )

                    # top-2 gating in [tokens, E] layout
                    rm = sp.tile([P, 1], F32, tag="rm", name=f"rm{sb}")
                    nc.vector.tensor_reduce(rm[:], score_ps[:], axis=AX.X, op=ALU.max, negate=True)
                    probs = sp.tile([P, E], F32, tag="probs", name=f"probs{sb}")
                    sume = sp.tile([P, 1], F32, tag="sume", name=f"sume{sb}")
                    nc.scalar.activation(probs[:], score_ps[:], AF.Exp, bias=rm[:], accum_out=sume[:])
                    rcp = sp.tile([P, 1], F32, tag="rcp", name=f"rcp{sb}")
                    nc.vector.reciprocal(rcp[:], sume[:])
                    pn = sp.tile([P, E], F32, tag="pn", name=f"pn{sb}")
                    nc.vector.tensor_scalar_mul(pn[:], probs[:], rcp[:])
                    p1 = sp.tile([P, 1], F32, tag="p1", name=f"p1{sb}")
                    nc.vector.tensor_reduce(p1[:], pn[:], axis=AX.X, op=ALU.max)
                    oh1 = sp.tile([P, E], F32, tag="oh1", name=f"oh1{sb}")
                    nc.vector.tensor_scalar(oh1[:], pn[:], p1[:], None, op0=ALU.is_equal)
                    pm = sp.tile([P, E], F32, tag="pm", name=f"pm{sb}")
                    nc.vector.tensor_tensor(pm[:], pn[:], oh1[:], ALU.subtract)
                    p2 = sp.tile([P, 1], F32, tag="p2", name=f"p2{sb}")
                    nc.vector.tensor_reduce(p2[:], pm[:], axis=AX.X, op=ALU.max)
                    oh2 = sp.tile([P, E], F32, tag="oh2", name=f"oh2{sb}")
                    nc.vector.tensor_scalar(oh2[:], pm[:], p2[:], None, op0=ALU.is_equal)
                    e1 = sp.tile([P, 1], F32, tag="e1", name=f"e1{sb}")
                    nc.scalar.activation(e1[:], p1[:], AF.Exp)
                    e2 = sp.tile([P, 1], F32, tag="e2", name=f"e2{sb}")
                    nc.scalar.activation(e2[:], p2[:], AF.Exp)
                    s12 = sp.tile([P, 1], F32, tag="s12", name=f"s12{sb}")
                    nc.vector.tensor_tensor(s12[:], e1[:], e2[:], ALU.add)
                    r12 = sp.tile([P, 1], F32, tag="r12", name=f"r12{sb}")
                    nc.vector.reciprocal(r12[:], s12[:])
                    w1 = sp.tile([P, 1], F32, tag="w1", name=f"w1{sb}")
                    nc.vector.tensor_scalar(w1[:], e1[:], r12[:], 2.0, op0=ALU.mult, op1=ALU.mult)
                    w2 = sp.tile([P, 1], F32, tag="w2", name=f"w2{sb}")
                    nc.vector.tensor_scalar(w2[:], e2[:], r12[:], 2.0, op0=ALU.mult, op1=ALU.mult)
                    cw_t = sp.tile([P, E], F32, tag="cw", name=f"cw{sb}")
                    nc.vector.tensor_scalar_mul(cw_t[:], oh1[:], w1[:])
                    t2 = sp.tile([P, E], F32, tag="t2", name=f"t2{sb}")
                    nc.vector.tensor_scalar_mul(t2[:], oh2[:], w2[:])
                    nc.vector.tensor_tensor(cw_t[:], cw_t[:], t2[:], ALU.add)
                    dm_b = sp.tile([P, E], BF16, tag="dmb", name=f"dmb{sb}")
                    nc.vector.tensor_tensor(dm_b[:], oh1[:], oh2[:], ALU.add)
                    cw_b = sp.tile([P, E], BF16, tag="cwb", name=f"cwb{sb}")
                    nc.vector.tensor_copy(cw_b[:], cw_t[:])

                    if pend is not None:
                        pend()

                    def make_pend(sb=sb, cw_b=cw_b, dm_b=dm_b, x_bf=x_bf):
                        def emit():
                            # cw^T (bf16) into [E, S] layout for combine
                            pc = psA.tile([P, P], BF16, tag="pst", bufs=3, name=f"pc{sb}")
                            nc.tensor.transpose(pc[:E, :], cw_b[:], ident_b[:])
                            nc.vector.tensor_copy(cwT_sb[:, sb, :], pc[:E, :])
                            # dispatch: D[e,m] += dm[s,e]^T @ x[s,m]
                            for c in range(DC):
                                nc.tensor.matmul(
                                    d_ps[:, ts(c, DCH)],
                                    lhsT=dm_b[:],
                                    rhs=x_bf[:, ts(c, DCH)],
                                    start=(sb == 0), stop=(sb == SB - 1),
                                )
                        return emit

                    pend = make_pend()
                pend()

            # dispatch AllToAll: row e -> core e; receive [G, M] for my expert.
            # Whole path in bf16 (the FFN consumes bf16 anyway) to shorten the
            # serial exchange chain; loads split in halves so the transposes
            # overlap the second half's arrival.
            d_sb = keep_pool.tile([E, M], BF16, name="d_sb")
            nc.vector.tensor_copy(d_sb[:, 0:M // 2], d_ps[:, 0:M // 2])
            nc.scalar.copy(d_sb[:, M // 2:], d_ps[:, M // 2:])
            nc.gpsimd.dma_start(d_in[:], d_sb[:])
            nc.gpsimd.collective_compute(
                "AllToAll", ALU.bypass, replica_groups=rg,
                ins=[d_in.opt()], outs=[d_out.opt()],
            )
            de_bf = keep_pool.tile([E, M], BF16, name="de_bf")
            nc.gpsimd.dma_start(de_bf[:, 0:M // 2], d_out[:, 0:M // 2])
            nc.gpsimd.dma_start(de_bf[:, M // 2:], d_out[:, M // 2:])
            for mo in range(MO):
                pd = psA.tile([P, E], BF16, tag="pst", bufs=3, name=f"pd{mo}")
                nc.tensor.transpose(pd[:], de_bf[:, ts(mo, P)], ident_b[:E, :E])
                nc.vector.tensor_copy(dt_sb[:, mo, :], pd[:])

        # ---------- phase B: expert FFN + combine (expert-parallel) ----------
        with (
            tc.tile_pool(name="wo", bufs=5) as wo_pool,
            tc.tile_pool(name="sp2", bufs=3) as sp2,
            tc.tile_pool(name="outp", bufs=4) as outp,
            tc.tile_pool(name="psB", bufs=2, space="PSUM") as psB,
            tc.tile_pool(name="psC", bufs=2, space="PSUM") as psC,
        ):
            # Pre-issue the first FFN2 chunk's wo loads on the Activation
            # queue: they fill the DMA gap while the dispatch AllToAll blocks
            # FFN1 (nc.sync is stalled behind WAR-blocked wi issues then).
            wo_tiles0 = []
            for q in range(NQ + 1):
                wo_t = wo_pool.tile([P, WOQ * MCH], BF16, tag="wo", name=f"wo0_{q}")
                nc.scalar.dma_start(wo_t[:], woP[:, ts(q, WOQ * MCH)])
                wo_tiles0.append(wo_t)

            # FFN1: h^T[h,g] = wiT[m,h]^T @ D^T[m,g], then silu -> ht_sb.
            # FFN2's first chunk (mc=0) rides inside this loop: quarter q only
            # needs ht blocks 16q..16q+15 (FFN1 chunks 4q..4q+3), its wo tile
            # is preloaded, and the PE has slack while FFN1 is DMA-bound —
            # this releases wo pool slots early so the wo stream never gaps
            # at the FFN1->FFN2 boundary.
            dmy1 = psB.tile([E, E], F32, tag="psh", name="dmy1")
            nc.tensor.matmul(dmy1[:], lhsT=dt_sb[:, MO - 1, :], rhs=dt_sb[:, MO - 1, :], start=True, stop=True)
            ps_eo0 = psB.tile([E, MCH], F32, tag="pseo", bufs=1, name="pseo0")
            hpend = None
            for hc in range(HC):
                wi_t = wi_pool.tile([P, MO * HCH], BF16, tag="wi", name=f"wi{hc}")
                nc.sync.dma_start(wi_t[:], wiP[:, ts(hc, MO * HCH)])
                ps_h = psB.tile([E, HCH], F32, tag="psh", name=f"psh{hc}")
                for mo in range(MO):
                    nc.tensor.matmul(
                        ps_h[:], lhsT=dt_sb[:, mo, :],
                        rhs=wi_t[:, ts(mo, HCH)],
                        start=(mo == 0), stop=(mo == MO - 1),
                    )
                # silu: sigmoid on ACT + multiply on DVE, both reading PSUM
                sg = sp2.tile([E, HCH], F32, tag="sg", bufs=2, name=f"sg{hc}")
                nc.scalar.activation(sg[:], ps_h[:], AF.Sigmoid)
                h_sb = sp2.tile([E, HCH], BF16, tag="hsb", name=f"h{hc}")
                nc.vector.tensor_tensor(h_sb[:], sg[:], ps_h[:], ALU.mult)
                if hpend is not None:
                    hpend()

                def make_hpend(hc=hc, h_sb=h_sb):
                    def emit():
                        for j in range(HCH // P):
                            pht = psB.tile([P, E], BF16, tag="psht", name=f"pht{hc}_{j}")
                            nc.tensor.transpose(pht[:], h_sb[:, ts(j, P)], ident_b[:E, :E])
                            nc.vector.tensor_copy(ht_sb[:, hc * (HCH // P) + j, :], pht[:])
                    return emit

                hpend = make_hpend()
                if hc % 4 == 3 and hc >= 7:
                    # FFN2 mc=0 quarter q = hc//4 - 1 (ht blocks ready, one
                    # FFN1 chunk of lag for the deferred pht transposes)
                    q = hc // 4 - 1
                    for hol in range(WOQ):
                        ho = q * WOQ + hol
                        nc.tensor.matmul(
                            ps_eo0[:], lhsT=ht_sb[:, ho, :],
                            rhs=wo_tiles0[q][:, ts(hol, MCH)],
                            start=(ho == 0), stop=False,
                        )
            hpend()
            # last mc=0 quarter (q=3) after FFN1 drains
            for hol in range(WOQ):
                ho = 3 * WOQ + hol
                nc.tensor.matmul(
                    ps_eo0[:], lhsT=ht_sb[:, ho, :],
                    rhs=wo_tiles0[3][:, ts(hol, MCH)],
                    start=False, stop=(ho == HB - 1),
                )
            nc.vector.tensor_copy(eo_sb[:, 0:MCH], ps_eo0[:])

            # One eo AllToAll per 512-column chunk: the collective device is
            # separate from the DMA engines, so four small exchanges cost
            # nothing extra there, and each chunk's combine (matmul + the
            # PSUM->SBUF copies, which are the expensive part) hides under the
            # still-running wo stream instead of piling up in the tail.
            eoall = [
                sp2.tile([E, MCH], BF16, tag="eoall", bufs=MC, name=f"eoall{i}")
                for i in range(MC)
            ]

            def emit_exchange(mc):
                nc.gpsimd.dma_start(eo_in[mc][:], eo_sb[:, ts(mc, MCH)])
                nc.gpsimd.collective_compute(
                    "AllToAll", ALU.bypass, replica_groups=rg,
                    ins=[eo_in[mc].opt()], outs=[eo_out[mc].opt()],
                )
                nc.gpsimd.dma_start(eoall[mc][:], eo_out[mc][:])

            # out viewed as [p, sb, m] so four row-blocks store in one DMA
            outV = out.rearrange("(b p) m -> p b m", p=P)

            def emit_combine(u):
                for g4 in range(SB // 4):
                    o_quad = outp.tile([P, 4, MCH], BF16, tag="osb", bufs=2, name=f"o{u}_{g4}")
                    for i in range(4):
                        sb = g4 * 4 + i
                        ps_o = psC.tile([P, MCH], F32, tag="pso", bufs=3, name=f"pso{u}_{sb}")
                        nc.tensor.matmul(
                            ps_o[:], lhsT=cwT_sb[:, sb, :], rhs=eoall[u][:],
                            start=True, stop=True,
                        )
                        if sb % 2 == 0:
                            nc.vector.tensor_copy(o_quad[:, i, :], ps_o[:])
                        else:
                            nc.scalar.copy(o_quad[:, i, :], ps_o[:])
                    # one batched 512 KB store per 4 row-blocks. Units 0-2
                    # store from the Activation queue so the sync FIFO (still
                    # streaming wo) is never delayed; the tail unit uses the
                    # by-then-idle sync queue.
                    eng = nc.scalar if u < MC - 1 else nc.sync
                    eng.dma_start(outV[:, g4 * 4:(g4 + 1) * 4, ts(u, MCH)], o_quad[:])

            emit_exchange(0)
            for mc in range(1, MC):
                ps_eo = psB.tile([E, MCH], F32, tag="psh", name=f"pseo{mc}")
                for q in range(NQ):
                    last_q = mc == MC - 1 and q == NQ - 1
                    if mc * NQ + q < len(wo_tiles0):
                        wo_ts = [wo_tiles0[mc * NQ + q]]
                    elif not last_q:
                        wo_t = wo_pool.tile([P, WOQ * MCH], BF16, tag="wo", name=f"wo{mc}_{q}")
                        nc.sync.dma_start(wo_t[:], woP[:, ts(mc * NQ + q, WOQ * MCH)])
                        wo_ts = [wo_t]
                    else:
                        # final quarter in two 1 MB halves so the very last
                        # FFN2 matmuls start half a quarter earlier
                        wo_ts = []
                        for hlf in range(2):
                            wo_t = wo_pool.tile([P, WOQ * MCH // 2], BF16, tag="wo", name=f"wo{mc}_{q}_{hlf}")
                            nc.sync.dma_start(
                                wo_t[:],
                                woP[:, (mc * NQ + q) * WOQ * MCH + hlf * (WOQ * MCH // 2):(mc * NQ + q) * WOQ * MCH + (hlf + 1) * (WOQ * MCH // 2)],
                            )
                            wo_ts.append(wo_t)
                    for hol in range(WOQ):
                        ho = q * WOQ + hol
                        wt = wo_ts[0] if hol < WOQ // len(wo_ts) * 1 and len(wo_ts) == 1 or (len(wo_ts) == 2 and hol < WOQ // 2) else wo_ts[-1]
                        off = hol if len(wo_ts) == 1 else (hol if hol < WOQ // 2 else hol - WOQ // 2)
                        nc.tensor.matmul(
                            ps_eo[:], lhsT=ht_sb[:, ho, :], rhs=wt[:, ts(off, MCH)],
                            start=(ho == 0), stop=(ho == HB - 1),
                        )
                emit_combine(mc - 1)
                if mc % 2 == 0:
                    nc.vector.tensor_copy(eo_sb[:, ts(mc, MCH)], ps_eo[:])
                else:
                    nc.scalar.copy(eo_sb[:, ts(mc, MCH)], ps_eo[:])
                emit_exchange(mc)
            emit_combine(MC - 1)

    nc.finalize()
    return nc


def prepare_in_maps(x, Wg, bg, wi, wo):
    G, S, M = x.shape
    E, H, _ = wi.shape
    MO, HB = M // P, H // P
    HCH, MCH = 512, 512
    HC, MC = H // HCH, M // MCH
    wg_arr = np.ascontiguousarray(
        np.asarray(Wg, dtype=np.float32).reshape(MO, P, E).transpose(1, 0, 2)
    )
    bg_arr = np.ascontiguousarray(np.asarray(bg, dtype=np.float32).reshape(1, E))
    in_maps = []
    for c in range(N_CORES):
        # wiP[p, hc, mo, c] = wi[e][hc*HCH+c, mo*P+p]
        wiP_c = np.ascontiguousarray(
            wi[c].reshape(HC, HCH, MO, P).transpose(3, 0, 2, 1).reshape(P, -1)
        ).astype(ml_dtypes.bfloat16)
        # woP[p, mc, ho, c] = wo[e][mc*MCH+c, ho*P+p]
        woP_c = np.ascontiguousarray(
            wo[c].reshape(MC, MCH, HB, P).transpose(3, 0, 2, 1).reshape(P, -1)
        ).astype(ml_dtypes.bfloat16)
        xc = np.ascontiguousarray(x[c], dtype=np.float32)
        in_maps.append({
            "xg": xc,
            "wg": wg_arr,
            "bgp": bg_arr,
            "wiP": wiP_c,
            "woP": woP_c,
        })
    return in_maps


def kernel(x, Wg, bg, wi, wo):
    global LAST_RESULT
    from concourse.bass_utils import run_bass_kernel_spmd

    x = np.asarray(x); Wg = np.asarray(Wg); bg = np.asarray(bg)
    wi = np.asarray(wi); wo = np.asarray(wo)
    nc = build_bass()
    in_maps = prepare_in_maps(x, Wg, bg, wi, wo)
    try:
        res = run_bass_kernel_spmd(
            nc, in_maps, core_ids=list(range(N_CORES)),
            trace=bool(int(os.environ.get("MOE_TRACE", "0"))),
        )
    except ModuleNotFoundError:
        # NTFF profiling hook unavailable in this environment — run untraced.
        os.environ["BASS_NEVER_TRACE"] = "1"
        res = run_bass_kernel_spmd(nc, in_maps, core_ids=list(range(N_CORES)))
    LAST_RESULT = res
    out = np.stack([r["out"] for r in res.results]).astype(np.float32)
    return out
